# revision 1
# baseline (speedup 1.0000x reference)
"""Trainium2 Bass kernel for CustomCenterQuantizerLinear.

Computes out = x @ f(weight_q).T + bias over 8 NeuronCores, where f is the
piecewise dequantizer:
    y = q / scale
    f = sign(y) * (eps + |y|*(gam-eps))        for |y| <= 1
    f = sign(y) * gam * exp(|y| - 1)           for |y| >  1
    f = 0                                      for y == 0

Sharding: tensor-parallel column split of weight/bias over out_features
(1024 per core), x replicated.

Branch-free reformulation (exact for integer q, which randint guarantees):
work in scaled units f' = f/alpha with alpha=(gam-eps)/scale, K=eps/alpha,
G=gam/alpha, B0=ln(G)-1:
    Ep = exp( q/scale + B0),  En = exp(-q/scale + B0)
    f'(q) = clamp(q + K*clamp(q,-1,1), -G, G) + max(Ep,G) - max(En,G)
The two max() terms become two PSUM-accumulated matmul streams (the second
negated on-chip), so no tensor-tensor select is ever needed; alpha is folded
into x on the host.
"""

import math
import sys

sys.path.insert(0, "/opt/trn_rl_repo")

import numpy as np
from ml_dtypes import bfloat16

B, S, IN, OUT = 8, 32, 8192, 8192
N_CORES = 8
M = B * S                 # 256 tokens
O_SH = OUT // N_CORES     # 1024 out features per core
KB = 128                  # contraction block (PE partition dim)
NKB = IN // KB            # 64 k-blocks
MB = 128                  # token block (PSUM partition dim)
NMB = M // MB             # 2 token blocks
OC = 512                  # matmul free-dim chunk (one PSUM bank)
NOC = O_SH // OC          # 2 chunks

_CACHE = {}


def _build(inv_s, b0, k_sign, g):
    import concourse.bass as bass
    import concourse.bacc as bacc
    import concourse.mybir as mybir
    import concourse.tile as tile

    BF = mybir.dt.bfloat16
    F32 = mybir.dt.float32
    Alu = mybir.AluOpType
    Act = mybir.ActivationFunctionType

    nc = bacc.Bacc("TRN2", target_bir_lowering=False, debug=False,
                   num_devices=N_CORES)
    wT_d = nc.dram_tensor("wT", [IN, O_SH], BF, kind="ExternalInput").ap()
    xT_d = nc.dram_tensor("xT", [KB, NKB * M], BF, kind="ExternalInput").ap()
    bias_d = nc.dram_tensor("bias", [1, O_SH], BF, kind="ExternalInput").ap()
    out_d = nc.dram_tensor("out", [M, O_SH], F32, kind="ExternalOutput").ap()

    with tile.TileContext(nc) as tc:
        with (
            tc.tile_pool(name="misc", bufs=1) as misc,
            tc.tile_pool(name="wp", bufs=4) as wp,
            tc.tile_pool(name="dq", bufs=4) as dq,
            tc.tile_pool(name="psum", bufs=1, space=bass.MemorySpace.PSUM) as pp,
        ):
            xT_sb = misc.tile([KB, NKB * M], BF)
            bias_sb = misc.tile([1, O_SH], BF)
            ones_sb = misc.tile([1, MB], BF)
            b0c = misc.tile([128, 1], F32)
            nc.gpsimd.dma_start(xT_sb[:], xT_d[:])
            nc.gpsimd.dma_start(bias_sb[:], bias_d[:])
            nc.vector.memset(ones_sb[:], 1.0)
            nc.vector.memset(b0c[:], b0)

            psums = [pp.tile([MB, O_SH], F32, name=f"ps{mi}", tag=f"ps{mi}")
                     for mi in range(NMB)]

            U16 = mybir.dt.uint16
            kbits = int(np.asarray(k_sign, dtype=bfloat16).view(np.uint16))
            NH = 4                 # k-blocks per dequant tile
            W2 = NH * O_SH
            for kp in range(NKB // NH):
                wt = wp.tile([KB, W2], BF)
                for h in range(NH):
                    kb = NH * kp + h
                    nc.sync.dma_start(
                        wt[:, h * O_SH:(h + 1) * O_SH],
                        wT_d[kb * KB:(kb + 1) * KB, :])

                ep = dq.tile([KB, W2], BF)
                en = dq.tile([KB, W2], BF)
                t3 = dq.tile([KB, W2], BF)

                nc.scalar.activation(ep[:], wt[:], Act.Exp,
                                     bias=b0c[:], scale=inv_s)
                nc.scalar.activation(en[:], wt[:], Act.Exp,
                                     bias=b0c[:], scale=-inv_s)
                # t3 = copysign(K, w): one 4x-mode pass via bit ops
                nc.vector.tensor_scalar(t3[:].bitcast(U16), wt[:].bitcast(U16),
                                        0x8000, kbits,
                                        Alu.bitwise_and, Alu.bitwise_or)
                # in-place chain: t3 <- u <- a2;  ep <- mep;  en <- r3n
                nc.vector.tensor_add(t3[:], wt[:], t3[:])
                nc.vector.tensor_scalar(t3[:], t3[:], -g, g, Alu.max, Alu.min)
                nc.vector.tensor_scalar(ep[:], ep[:], g, None, Alu.max)
                nc.vector.tensor_add(ep[:], ep[:], t3[:])
                nc.vector.tensor_scalar(en[:], en[:], g, -1.0,
                                        Alu.max, Alu.mult)
                f1, r3n = ep, en

                for h in range(NH):
                    kb = NH * kp + h
                    for mi in range(NMB):
                        lhsT = xT_sb[:, kb * M + mi * MB:
                                     kb * M + (mi + 1) * MB]
                        for oc in range(NOC):
                            sl = slice(h * O_SH + oc * OC,
                                       h * O_SH + (oc + 1) * OC)
                            psl = slice(oc * OC, (oc + 1) * OC)
                            nc.tensor.matmul(psums[mi][:, psl], lhsT,
                                             f1[:, sl],
                                             start=(kb == 0), stop=False)
                            nc.tensor.matmul(psums[mi][:, psl], lhsT,
                                             r3n[:, sl],
                                             start=False, stop=False)

            for mi in range(NMB):
                for oc in range(NOC):
                    sl = slice(oc * OC, (oc + 1) * OC)
                    nc.tensor.matmul(psums[mi][:, sl], ones_sb[:],
                                     bias_sb[:, sl], start=False, stop=True)

            for mi in range(NMB):
                osb = misc.tile([MB, O_SH], F32, name=f"osb{mi}",
                                tag=f"osb{mi}")
                nc.scalar.copy(osb[:], psums[mi][:])
                nc.sync.dma_start(out_d[mi * MB:(mi + 1) * MB, :], osb[:])

    nc.compile()
    return nc


def _get_nc(inv_s, b0, k_sign, g):
    key = (round(inv_s, 12), round(b0, 12), round(k_sign, 12), round(g, 12))
    if key not in _CACHE:
        _CACHE[key] = _build(inv_s, b0, k_sign, g)
    return _CACHE[key]


def _prep_inputs(x, epsilon, gamma, scale, bias, weight_q):
    eps = float(np.asarray(epsilon).ravel()[0])
    gam = float(np.asarray(gamma).ravel()[0])
    sc = float(np.asarray(scale).ravel()[0])
    alpha = (gam - eps) / sc
    assert alpha > 0
    k_sign = eps / alpha
    g = gam / alpha
    b0 = math.log(g) - 1.0
    inv_s = 1.0 / sc

    xr = np.asarray(x, dtype=np.float32).reshape(M, IN) * np.float32(alpha)
    xT = np.ascontiguousarray(xr.T)                       # [IN, M]
    xT_blocked = np.ascontiguousarray(
        xT.reshape(NKB, KB, M).transpose(1, 0, 2)
    ).reshape(KB, NKB * M).astype(bfloat16)

    wbf = np.asarray(weight_q).astype(bfloat16)           # exact: |q| <= 127
    bias_bf = np.asarray(bias, dtype=np.float32).astype(bfloat16)

    in_maps = []
    for c in range(N_CORES):
        wTc = np.ascontiguousarray(
            wbf[c * O_SH:(c + 1) * O_SH, :].T)            # [IN, O_SH]
        in_maps.append({
            "wT": wTc,
            "xT": xT_blocked,
            "bias": bias_bf[c * O_SH:(c + 1) * O_SH].reshape(1, O_SH),
        })
    return (inv_s, b0, k_sign, g), in_maps


def _run(nc, in_maps, **kw):
    from concourse import bass_utils
    return bass_utils.run_bass_kernel_spmd(
        nc, in_maps, core_ids=list(range(N_CORES)), **kw)


def kernel(x, epsilon, gamma, scale, bias, weight_q):
    consts, in_maps = _prep_inputs(x, epsilon, gamma, scale, bias, weight_q)
    nc = _get_nc(*consts)
    res = _run(nc, in_maps)
    out = np.concatenate(
        [np.asarray(res.results[c]["out"]) for c in range(N_CORES)], axis=1)
    return np.ascontiguousarray(out.reshape(B, S, OUT)).astype(np.float32)



# revision 31
# speedup vs baseline: 1.8661x; 1.8661x over previous
"""Trainium2 Bass kernel for CustomCenterQuantizerLinear.

Computes out = x @ f(weight_q).T + bias over 8 NeuronCores, where f is the
piecewise dequantizer:
    y = q / scale
    f = sign(y) * (eps + |y|*(gam-eps))        for |y| <= 1
    f = sign(y) * gam * exp(|y| - 1)           for |y| >  1
    f = 0                                      for y == 0

Sharding: tensor-parallel column split of weight/bias over out_features
(1024 per core), x replicated.

Device math (exact, in alpha-units with alpha=(gam-eps)/scale, K=eps/alpha,
G=gam/alpha = K+scale): the host un-centers the integer codes,
    d = q + K*sign(q)            (0 -> 0; |d| = |q|+K in [K+1, K+127])
and the device evaluates the multiplicative form
    core = clamp(d, -G, G)                    # signed, = sgn*min(|q|+K, G)
    rd   = max(|d|, G) - G                    # = relu(|q| - scale)
    e    = exp(rd / scale)                    # = 1 in-range, e^(|y|-1) in tail
    f    = core * e                           # sign carried by the multiply
which matches f/alpha exactly on all integer codes (both branches agree at
the |q|=scale breakpoint because clamp and relu share it). alpha is folded
into x on the host.

Per 4-k-block tile [128, 4096] this costs two 4x-mode tensor_scalar passes
plus one 2x tensor_tensor on DVE, one Exp pass on Act, and a single
PSUM-accumulated matmul stream on PE; a slice of the relu pass runs on the
otherwise-idle Pool engine to keep DVE at the DMA roofline.
"""

import sys

sys.path.insert(0, "/opt/trn_rl_repo")

import numpy as np
from ml_dtypes import bfloat16

B, S, IN, OUT = 8, 32, 8192, 8192
N_CORES = 8
M = B * S                 # 256 tokens
O_SH = OUT // N_CORES     # 1024 out features per core
KB = 128                  # contraction block (PE partition dim)
NKB = IN // KB            # 64 k-blocks
MB = 128                  # token block (PSUM partition dim)
NMB = M // MB             # 2 token blocks
OC = 512                  # matmul free-dim chunk (one PSUM bank)
NOC = O_SH // OC          # 2 chunks
NH = 4                    # k-blocks per dequant tile
W2 = NH * O_SH            # dequant tile width (4096)
POOL_COLS = 2688          # columns of the clamp pass offloaded to Pool
OUT_BF16 = True           # evict PSUM to bf16 (halves output DMA)
DIRECT_EVICT = False      # DMA PSUM -> HBM directly (skips Act copy)
ACT_HALVES = 2            # exp pass split factor
TT_HALVES = 2             # mult pass split factor
WP_BUFS = 3               # weight-tile double buffering depth
DQ_BUFS = 4               # dequant-tile pool depth
F_ALIAS = False           # write f in-place over rd

_CACHE = {}


def _build(inv_s, g):
    import concourse.bass as bass
    import concourse.bacc as bacc
    import concourse.mybir as mybir
    import concourse.tile as tile

    BF = mybir.dt.bfloat16
    F32 = mybir.dt.float32
    U16 = mybir.dt.uint16
    Alu = mybir.AluOpType
    Act = mybir.ActivationFunctionType
    gbits = int(np.asarray(g, dtype=bfloat16).view(np.uint16))

    nc = bacc.Bacc("TRN2", target_bir_lowering=False, debug=False,
                   num_devices=N_CORES)
    ODT = BF if OUT_BF16 else F32
    wT_d = nc.dram_tensor("wT", [IN, O_SH], BF, kind="ExternalInput").ap()
    xT_d = nc.dram_tensor("xT", [KB, NKB * M], BF, kind="ExternalInput").ap()
    bias_d = nc.dram_tensor("bias", [1, O_SH], BF, kind="ExternalInput").ap()
    out_d = nc.dram_tensor("out", [M, O_SH], ODT, kind="ExternalOutput").ap()

    DC = W2 - POOL_COLS       # columns of the relu pass kept on DVE

    with tile.TileContext(nc) as tc:
        with (
            tc.tile_pool(name="misc", bufs=1) as misc,
            tc.tile_pool(name="wp", bufs=WP_BUFS) as wp,
            tc.tile_pool(name="dq", bufs=DQ_BUFS) as dq,
            tc.tile_pool(name="psum", bufs=1, space=bass.MemorySpace.PSUM) as pp,
        ):
            xT_sb = misc.tile([KB, NKB * M], BF)
            bias_sb = misc.tile([1, O_SH], BF)
            ones_sb = misc.tile([1, MB], BF)
            nc.gpsimd.dma_start(bias_sb[:], bias_d[:])
            nc.vector.memset(ones_sb[:], 1.0)
            b0c = misc.tile([128, 1], F32)
            nc.vector.memset(b0c[:], 0.0)

            psums = [pp.tile([MB, O_SH], F32, name=f"ps{mi}", tag=f"ps{mi}")
                     for mi in range(NMB)]

            # seed the accumulators with the bias so the tail has no extra
            # matmul round: psum = ones^T @ bias, start=True
            for mi in range(NMB):
                for oc in range(NOC):
                    sl = slice(oc * OC, (oc + 1) * OC)
                    nc.tensor.matmul(psums[mi][:, sl], ones_sb[:],
                                     bias_sb[:, sl], start=True, stop=False)

            XCH = NH * M         # x columns consumed per kp iteration
            for kp in range(NKB // NH):
                wt = wp.tile([KB, W2], BF)
                for h in range(NH):
                    kb = NH * kp + h
                    nc.sync.dma_start(
                        wt[:, h * O_SH:(h + 1) * O_SH],
                        wT_d[kb * KB:(kb + 1) * KB, :])
                # x arrives just-in-time, one kp-slice behind the weights,
                # so the head of the pipeline isn't gated on the full 4.2MB
                nc.sync.dma_start(xT_sb[:, kp * XCH:(kp + 1) * XCH],
                                  xT_d[:, kp * XCH:(kp + 1) * XCH])

                ct = dq.tile([KB, W2], BF)
                rd = dq.tile([KB, W2], BF)
                e3 = dq.tile([KB, W2], BF)
                f = rd if F_ALIAS else dq.tile([KB, W2], BF)

                # rd = relu(|d| - G), the tail excess, feeding the Exp pass.
                # DVE can't pair the bitwise sign-strip with an arith max in
                # one instruction, so this is two passes.
                nc.vector.tensor_scalar(rd[:].bitcast(U16),
                                        wt[:].bitcast(U16), 0x7FFF,
                                        None, Alu.bitwise_and)
                nc.vector.tensor_scalar(rd[:], rd[:], g, g,
                                        Alu.max, Alu.subtract)
                # core = clamp(d, -G, G); off the exp critical path
                nc.vector.tensor_scalar(ct[:], wt[:], -g, g,
                                        Alu.max, Alu.min)
                # e = exp(rd / scale) >= 1, then f = core * e (sign rides
                # on core; e > 0) — both in column slices so downstream
                # stages start before the full tile is done
                AW = W2 // ACT_HALVES
                for hh in range(ACT_HALVES):
                    hs = slice(hh * AW, (hh + 1) * AW)
                    nc.scalar.activation(e3[:, hs], rd[:, hs], Act.Exp,
                                         bias=b0c[:], scale=inv_s)
                # merge f = core * e; Pool (GPSIMD Multiply) takes the last
                # POOL_COLS columns, DVE the rest in TT_HALVES slices
                if POOL_COLS:
                    nc.gpsimd.tensor_tensor(f[:, DC:], ct[:, DC:],
                                            e3[:, DC:], Alu.mult)
                TW = DC // TT_HALVES
                for hh in range(TT_HALVES):
                    hs = slice(hh * TW, (hh + 1) * TW)
                    nc.vector.tensor_tensor(f[:, hs], ct[:, hs],
                                            e3[:, hs], Alu.mult)

                last = kp == NKB // NH - 1
                # mi-major on the final tile so psum mi=0 closes (and can
                # evict) while mi=1's matmuls still run
                if last:
                    order = [(h, mi) for mi in range(NMB) for h in range(NH)]
                else:
                    order = [(h, mi) for h in range(NH) for mi in range(NMB)]
                for h, mi in order:
                    kb = NH * kp + h
                    lhsT = xT_sb[:, kb * M + mi * MB:
                                 kb * M + (mi + 1) * MB]
                    for oc in range(NOC):
                        sl = slice(h * O_SH + oc * OC,
                                   h * O_SH + (oc + 1) * OC)
                        psl = slice(oc * OC, (oc + 1) * OC)
                        nc.tensor.matmul(psums[mi][:, psl], lhsT,
                                         f[:, sl], start=False,
                                         stop=last and h == NH - 1)

            for mi in range(NMB):
                osb = misc.tile([MB, O_SH], ODT, name=f"osb{mi}",
                                tag=f"osb{mi}")
                if mi == 0:
                    nc.scalar.copy(osb[:], psums[mi][:])
                else:
                    nc.vector.tensor_scalar(osb[:], psums[mi][:], 0.0, None,
                                            Alu.add)
                nc.sync.dma_start(out_d[mi * MB:(mi + 1) * MB, :], osb[:])

    nc.compile()
    return nc


def _get_nc(inv_s, g):
    key = (round(inv_s, 12), round(g, 12))
    if key not in _CACHE:
        _CACHE[key] = _build(inv_s, g)
    return _CACHE[key]


def _prep_inputs(x, epsilon, gamma, scale, bias, weight_q):
    eps = float(np.asarray(epsilon).ravel()[0])
    gam = float(np.asarray(gamma).ravel()[0])
    sc = float(np.asarray(scale).ravel()[0])
    alpha = (gam - eps) / sc
    assert alpha > 0
    k_sign = eps / alpha
    g = gam / alpha
    inv_s = 1.0 / sc

    xr = np.asarray(x, dtype=np.float32).reshape(M, IN) * np.float32(alpha)
    xT = np.ascontiguousarray(xr.T)                       # [IN, M]
    xT_blocked = np.ascontiguousarray(
        xT.reshape(NKB, KB, M).transpose(1, 0, 2)
    ).reshape(KB, NKB * M).astype(bfloat16)

    # un-center the integer codes: d = q + K*sign(q), exact in bf16
    wq = np.asarray(weight_q, dtype=np.float32)
    wd = (wq + np.float32(k_sign) * np.sign(wq)).astype(bfloat16)
    bias_bf = np.asarray(bias, dtype=np.float32).astype(bfloat16)

    in_maps = []
    for c in range(N_CORES):
        wTc = np.ascontiguousarray(
            wd[c * O_SH:(c + 1) * O_SH, :].T)             # [IN, O_SH]
        in_maps.append({
            "wT": wTc,
            "xT": xT_blocked,
            "bias": bias_bf[c * O_SH:(c + 1) * O_SH].reshape(1, O_SH),
        })
    return (inv_s, g), in_maps


def _run(nc, in_maps, **kw):
    from concourse import bass_utils
    return bass_utils.run_bass_kernel_spmd(
        nc, in_maps, core_ids=list(range(N_CORES)), **kw)


def kernel(x, epsilon, gamma, scale, bias, weight_q):
    consts, in_maps = _prep_inputs(x, epsilon, gamma, scale, bias, weight_q)
    nc = _get_nc(*consts)
    res = _run(nc, in_maps)
    out = np.concatenate(
        [np.asarray(res.results[c]["out"]) for c in range(N_CORES)], axis=1)
    return np.ascontiguousarray(out.reshape(B, S, OUT)).astype(np.float32)


# revision 36
# speedup vs baseline: 1.9213x; 1.0295x over previous
"""Trainium2 Bass kernel for CustomCenterQuantizerLinear.

Computes out = x @ f(weight_q).T + bias over 8 NeuronCores, where f is the
piecewise dequantizer:
    y = q / scale
    f = sign(y) * (eps + |y|*(gam-eps))        for |y| <= 1
    f = sign(y) * gam * exp(|y| - 1)           for |y| >  1
    f = 0                                      for y == 0

Sharding: tensor-parallel column split of weight/bias over out_features
(1024 per core), x replicated.

Device math (exact, in alpha-units with alpha=(gam-eps)/scale, K=eps/alpha,
G=gam/alpha = K+scale): the host un-centers the integer codes,
    d = q + K*sign(q)            (0 -> 0; |d| = |q|+K in [K+1, K+127])
and the device evaluates the multiplicative form
    core = clamp(d, -G, G)                    # signed, = sgn*min(|q|+K, G)
    rd   = max(|d|, G) - G                    # = relu(|q| - scale)
    e    = exp(rd / scale)                    # = 1 in-range, e^(|y|-1) in tail
    f    = core * e                           # sign carried by the multiply
which matches f/alpha exactly on all integer codes (both branches agree at
the |q|=scale breakpoint because clamp and relu share it). alpha is folded
into x on the host.

Per 4-k-block tile [128, 4096] this costs two 4x-mode tensor_scalar passes
plus one 2x tensor_tensor on DVE, one Exp pass on Act, and a single
PSUM-accumulated matmul stream on PE; a slice of the relu pass runs on the
otherwise-idle Pool engine to keep DVE at the DMA roofline.
"""

import sys

sys.path.insert(0, "/opt/trn_rl_repo")

import numpy as np
from ml_dtypes import bfloat16

B, S, IN, OUT = 8, 32, 8192, 8192
N_CORES = 8
M = B * S                 # 256 tokens
O_SH = OUT // N_CORES     # 1024 out features per core
KB = 128                  # contraction block (PE partition dim)
NKB = IN // KB            # 64 k-blocks
MB = 128                  # token block (PSUM partition dim)
NMB = M // MB             # 2 token blocks
OC = 512                  # matmul free-dim chunk (one PSUM bank)
NOC = O_SH // OC          # 2 chunks
NH = 4                    # k-blocks per dequant tile
W2 = NH * O_SH            # dequant tile width (4096)
POOL_COLS = 2048          # columns of the merge offloaded to Pool (Multiply)
OUT_BF16 = True           # evict PSUM to bf16 (halves output DMA)
DIRECT_EVICT = False      # (unsupported: PSUM->HBM DMA rejected by bass)
ACT_HALVES = 4            # exp pass split factor
TT_HALVES = 2             # DVE merge split factor
POOL_HALVES = 2           # pool merge split factor
WP_BUFS = 3               # weight-tile double buffering depth
DQ_BUFS = 4               # dequant-tile pool depth
F_ALIAS = False           # write f in-place over rd

_CACHE = {}


def _build(inv_s, g):
    import concourse.bass as bass
    import concourse.bacc as bacc
    import concourse.mybir as mybir
    import concourse.tile as tile

    BF = mybir.dt.bfloat16
    F32 = mybir.dt.float32
    U16 = mybir.dt.uint16
    Alu = mybir.AluOpType
    Act = mybir.ActivationFunctionType
    gbits = int(np.asarray(g, dtype=bfloat16).view(np.uint16))

    nc = bacc.Bacc("TRN2", target_bir_lowering=False, debug=False,
                   num_devices=N_CORES)
    ODT = BF if OUT_BF16 else F32
    wT_d = nc.dram_tensor("wT", [IN, O_SH], BF, kind="ExternalInput").ap()
    xT_d = nc.dram_tensor("xT", [KB, NKB * M], BF, kind="ExternalInput").ap()
    bias_d = nc.dram_tensor("bias", [1, O_SH], BF, kind="ExternalInput").ap()
    out_d = nc.dram_tensor("out", [M, O_SH], ODT, kind="ExternalOutput").ap()

    DC = W2 - POOL_COLS       # columns of the relu pass kept on DVE

    with tile.TileContext(nc) as tc:
        with (
            tc.tile_pool(name="misc", bufs=1) as misc,
            tc.tile_pool(name="wp", bufs=WP_BUFS) as wp,
            tc.tile_pool(name="dq", bufs=DQ_BUFS) as dq,
            tc.tile_pool(name="psum", bufs=1, space=bass.MemorySpace.PSUM) as pp,
        ):
            xT_sb = misc.tile([KB, NKB * M], BF)
            bias_sb = misc.tile([1, O_SH], BF)
            ones_sb = misc.tile([1, MB], BF)
            nc.gpsimd.dma_start(bias_sb[:], bias_d[:])
            nc.vector.memset(ones_sb[:], 1.0)
            b0c = misc.tile([128, 1], F32)
            nc.vector.memset(b0c[:], 0.0)

            psums = [pp.tile([MB, O_SH], F32, name=f"ps{mi}", tag=f"ps{mi}")
                     for mi in range(NMB)]

            # seed the accumulators with the bias so the tail has no extra
            # matmul round: psum = ones^T @ bias, start=True
            for mi in range(NMB):
                for oc in range(NOC):
                    sl = slice(oc * OC, (oc + 1) * OC)
                    nc.tensor.matmul(psums[mi][:, sl], ones_sb[:],
                                     bias_sb[:, sl], start=True, stop=False)

            XCH = NH * M         # x columns consumed per kp iteration
            for kp in range(NKB // NH):
                wt = wp.tile([KB, W2], BF)
                for h in range(NH):
                    kb = NH * kp + h
                    nc.sync.dma_start(
                        wt[:, h * O_SH:(h + 1) * O_SH],
                        wT_d[kb * KB:(kb + 1) * KB, :])
                # x arrives just-in-time, one kp-slice behind the weights,
                # so the head of the pipeline isn't gated on the full 4.2MB
                nc.sync.dma_start(xT_sb[:, kp * XCH:(kp + 1) * XCH],
                                  xT_d[:, kp * XCH:(kp + 1) * XCH])

                ct = dq.tile([KB, W2], BF)
                rd = dq.tile([KB, W2], BF)
                e3 = dq.tile([KB, W2], BF)
                f = rd if F_ALIAS else dq.tile([KB, W2], BF)

                # rd = relu(|d| - G), the tail excess, feeding the Exp pass.
                # DVE can't pair the bitwise sign-strip with an arith max in
                # one instruction, so this is two passes.
                nc.vector.tensor_scalar(rd[:].bitcast(U16),
                                        wt[:].bitcast(U16), 0x7FFF,
                                        None, Alu.bitwise_and)
                nc.vector.tensor_scalar(rd[:], rd[:], g, g,
                                        Alu.max, Alu.subtract)
                # core = clamp(d, -G, G); Pool's merge columns first so its
                # Multiply isn't gated on the whole pass
                nc.vector.tensor_scalar(ct[:, :POOL_COLS], wt[:, :POOL_COLS],
                                        -g, g, Alu.max, Alu.min)
                nc.vector.tensor_scalar(ct[:, POOL_COLS:], wt[:, POOL_COLS:],
                                        -g, g, Alu.max, Alu.min)
                # e = exp(rd / scale) >= 1, then f = core * e (sign rides
                # on core; e > 0) — both in column slices so downstream
                # stages start before the full tile is done
                AW = W2 // ACT_HALVES
                for hh in range(ACT_HALVES):
                    hs = slice(hh * AW, (hh + 1) * AW)
                    nc.scalar.activation(e3[:, hs], rd[:, hs], Act.Exp,
                                         bias=b0c[:], scale=inv_s)
                # merge f = core * e; Pool (GPSIMD Multiply) takes the FIRST
                # POOL_COLS columns (their e3 lands first, so Pool starts
                # early), DVE the rest in TT_HALVES slices
                PW = POOL_COLS // POOL_HALVES if POOL_COLS else 0
                for hh in range(POOL_HALVES if POOL_COLS else 0):
                    hs = slice(hh * PW, (hh + 1) * PW)
                    nc.gpsimd.tensor_tensor(f[:, hs], ct[:, hs],
                                            e3[:, hs], Alu.mult)
                TW = DC // TT_HALVES
                for hh in range(TT_HALVES):
                    hs = slice(POOL_COLS + hh * TW, POOL_COLS + (hh + 1) * TW)
                    nc.vector.tensor_tensor(f[:, hs], ct[:, hs],
                                            e3[:, hs], Alu.mult)

                last = kp == NKB // NH - 1
                # mi-major on the final tile so psum mi=0 closes (and can
                # evict) while mi=1's matmuls still run
                if last:
                    order = [(h, mi) for mi in range(NMB) for h in range(NH)]
                else:
                    order = [(h, mi) for h in range(NH) for mi in range(NMB)]
                for h, mi in order:
                    kb = NH * kp + h
                    lhsT = xT_sb[:, kb * M + mi * MB:
                                 kb * M + (mi + 1) * MB]
                    for oc in range(NOC):
                        sl = slice(h * O_SH + oc * OC,
                                   h * O_SH + (oc + 1) * OC)
                        psl = slice(oc * OC, (oc + 1) * OC)
                        nc.tensor.matmul(psums[mi][:, psl], lhsT,
                                         f[:, sl], start=False,
                                         stop=last and h == NH - 1)

            for mi in range(NMB):
                osb = misc.tile([MB, O_SH], ODT, name=f"osb{mi}",
                                tag=f"osb{mi}")
                if mi == 0:
                    nc.scalar.copy(osb[:], psums[mi][:])
                else:
                    nc.vector.tensor_scalar(osb[:], psums[mi][:], 0.0, None,
                                            Alu.add)
                nc.sync.dma_start(out_d[mi * MB:(mi + 1) * MB, :], osb[:])

    nc.compile()
    return nc


def _get_nc(inv_s, g):
    key = (round(inv_s, 12), round(g, 12))
    if key not in _CACHE:
        _CACHE[key] = _build(inv_s, g)
    return _CACHE[key]


def _prep_inputs(x, epsilon, gamma, scale, bias, weight_q):
    eps = float(np.asarray(epsilon).ravel()[0])
    gam = float(np.asarray(gamma).ravel()[0])
    sc = float(np.asarray(scale).ravel()[0])
    alpha = (gam - eps) / sc
    assert alpha > 0
    k_sign = eps / alpha
    g = gam / alpha
    inv_s = 1.0 / sc

    xr = np.asarray(x, dtype=np.float32).reshape(M, IN) * np.float32(alpha)
    xT = np.ascontiguousarray(xr.T)                       # [IN, M]
    xT_blocked = np.ascontiguousarray(
        xT.reshape(NKB, KB, M).transpose(1, 0, 2)
    ).reshape(KB, NKB * M).astype(bfloat16)

    # un-center the integer codes: d = q + K*sign(q), exact in bf16
    wq = np.asarray(weight_q, dtype=np.float32)
    wd = (wq + np.float32(k_sign) * np.sign(wq)).astype(bfloat16)
    bias_bf = np.asarray(bias, dtype=np.float32).astype(bfloat16)

    in_maps = []
    for c in range(N_CORES):
        wTc = np.ascontiguousarray(
            wd[c * O_SH:(c + 1) * O_SH, :].T)             # [IN, O_SH]
        in_maps.append({
            "wT": wTc,
            "xT": xT_blocked,
            "bias": bias_bf[c * O_SH:(c + 1) * O_SH].reshape(1, O_SH),
        })
    return (inv_s, g), in_maps


def _run(nc, in_maps, **kw):
    from concourse import bass_utils
    return bass_utils.run_bass_kernel_spmd(
        nc, in_maps, core_ids=list(range(N_CORES)), **kw)


def kernel(x, epsilon, gamma, scale, bias, weight_q):
    consts, in_maps = _prep_inputs(x, epsilon, gamma, scale, bias, weight_q)
    nc = _get_nc(*consts)
    res = _run(nc, in_maps)
    out = np.concatenate(
        [np.asarray(res.results[c]["out"]) for c in range(N_CORES)], axis=1)
    return np.ascontiguousarray(out.reshape(B, S, OUT)).astype(np.float32)


# revision 41
# speedup vs baseline: 1.9333x; 1.0062x over previous
"""Trainium2 Bass kernel for CustomCenterQuantizerLinear.

Computes out = x @ f(weight_q).T + bias over 8 NeuronCores, where f is the
piecewise dequantizer:
    y = q / scale
    f = sign(y) * (eps + |y|*(gam-eps))        for |y| <= 1
    f = sign(y) * gam * exp(|y| - 1)           for |y| >  1
    f = 0                                      for y == 0

Sharding: tensor-parallel column split of weight/bias over out_features
(1024 per core), x replicated.

Device math (exact, in alpha-units with alpha=(gam-eps)/scale, K=eps/alpha,
G=gam/alpha = K+scale): the host un-centers the integer codes,
    d = q + K*sign(q)            (0 -> 0; |d| = |q|+K in [K+1, K+127])
and the device evaluates the multiplicative form
    core = clamp(d, -G, G)                    # signed, = sgn*min(|q|+K, G)
    rd   = max(|d|, G) - G                    # = relu(|q| - scale)
    e    = exp(rd / scale)                    # = 1 in-range, e^(|y|-1) in tail
    f    = core * e                           # sign carried by the multiply
which matches f/alpha exactly on all integer codes (both branches agree at
the |q|=scale breakpoint because clamp and relu share it). alpha is folded
into x on the host.

Per 4-k-block tile [128, 4096] this costs two 4x-mode tensor_scalar passes
plus one 2x tensor_tensor on DVE, one Exp pass on Act, and a single
PSUM-accumulated matmul stream on PE; a slice of the relu pass runs on the
otherwise-idle Pool engine to keep DVE at the DMA roofline.
"""

import sys

sys.path.insert(0, "/opt/trn_rl_repo")

import numpy as np
from ml_dtypes import bfloat16

B, S, IN, OUT = 8, 32, 8192, 8192
N_CORES = 8
M = B * S                 # 256 tokens
O_SH = OUT // N_CORES     # 1024 out features per core
KB = 128                  # contraction block (PE partition dim)
NKB = IN // KB            # 64 k-blocks
MB = 128                  # token block (PSUM partition dim)
NMB = M // MB             # 2 token blocks
OC = 512                  # matmul free-dim chunk (one PSUM bank)
NOC = O_SH // OC          # 2 chunks
NH = 4                    # k-blocks per dequant tile
W2 = NH * O_SH            # dequant tile width (4096)
POOL_COLS = 2048          # columns of the merge offloaded to Pool (Multiply)
OUT_BF16 = True           # evict PSUM to bf16 (halves output DMA)
DIRECT_EVICT = False      # (unsupported: PSUM->HBM DMA rejected by bass)
ACT_HALVES = 4            # exp pass split factor (must divide W2)
TT_HALVES = 1             # DVE merge split factor
POOL_HALVES = 2           # pool merge split factor
HEAD_FINE = True          # fine-grained first tile (shorter pipeline fill)
WP_BUFS = 3               # weight-tile double buffering depth
DQ_BUFS = 4               # dequant-tile pool depth
F_ALIAS = False           # write f in-place over rd

_CACHE = {}


def _build(inv_s, g):
    import concourse.bass as bass
    import concourse.bacc as bacc
    import concourse.mybir as mybir
    import concourse.tile as tile

    BF = mybir.dt.bfloat16
    F32 = mybir.dt.float32
    U16 = mybir.dt.uint16
    Alu = mybir.AluOpType
    Act = mybir.ActivationFunctionType
    gbits = int(np.asarray(g, dtype=bfloat16).view(np.uint16))

    nc = bacc.Bacc("TRN2", target_bir_lowering=False, debug=False,
                   num_devices=N_CORES)
    ODT = BF if OUT_BF16 else F32
    wT_d = nc.dram_tensor("wT", [IN, O_SH], BF, kind="ExternalInput").ap()
    xT_d = nc.dram_tensor("xT", [KB, NKB * M], BF, kind="ExternalInput").ap()
    bias_d = nc.dram_tensor("bias", [1, O_SH], BF, kind="ExternalInput").ap()
    out_d = nc.dram_tensor("out", [M, O_SH], ODT, kind="ExternalOutput").ap()

    DC = W2 - POOL_COLS       # columns of the relu pass kept on DVE

    with tile.TileContext(nc) as tc:
        with (
            tc.tile_pool(name="misc", bufs=1) as misc,
            tc.tile_pool(name="wp", bufs=WP_BUFS) as wp,
            tc.tile_pool(name="dq", bufs=DQ_BUFS) as dq,
            tc.tile_pool(name="psum", bufs=1, space=bass.MemorySpace.PSUM) as pp,
        ):
            xT_sb = misc.tile([KB, NKB * M], BF)
            bias_sb = misc.tile([1, O_SH], BF)
            ones_sb = misc.tile([1, MB], BF)
            nc.gpsimd.dma_start(bias_sb[:], bias_d[:])
            nc.vector.memset(ones_sb[:], 1.0)
            b0c = misc.tile([128, 1], F32)
            nc.vector.memset(b0c[:], 0.0)

            psums = [pp.tile([MB, O_SH], F32, name=f"ps{mi}", tag=f"ps{mi}")
                     for mi in range(NMB)]

            # seed the accumulators with the bias so the tail has no extra
            # matmul round: psum = ones^T @ bias, start=True
            for mi in range(NMB):
                for oc in range(NOC):
                    sl = slice(oc * OC, (oc + 1) * OC)
                    nc.tensor.matmul(psums[mi][:, sl], ones_sb[:],
                                     bias_sb[:, sl], start=True, stop=False)

            XCH = NH * M         # x columns consumed per kp iteration
            for kp in range(NKB // NH):
                wt = wp.tile([KB, W2], BF)
                if kp == 0 and HEAD_FINE:
                    # interleave the first x half between the first weight
                    # blocks so the h=0 matmuls aren't gated on 1MB+x
                    nc.sync.dma_start(wt[:, :O_SH], wT_d[:KB, :])
                    nc.sync.dma_start(xT_sb[:, :XCH // 2],
                                      xT_d[:, :XCH // 2])
                    for h in range(1, NH):
                        nc.sync.dma_start(wt[:, h * O_SH:(h + 1) * O_SH],
                                          wT_d[h * KB:(h + 1) * KB, :])
                    nc.sync.dma_start(xT_sb[:, XCH // 2:XCH],
                                      xT_d[:, XCH // 2:XCH])
                else:
                    for h in range(NH):
                        kb = NH * kp + h
                        nc.sync.dma_start(
                            wt[:, h * O_SH:(h + 1) * O_SH],
                            wT_d[kb * KB:(kb + 1) * KB, :])
                    # x arrives just-in-time, one kp-slice behind the
                    # weights, so the head isn't gated on the full 4.2MB
                    nc.sync.dma_start(xT_sb[:, kp * XCH:(kp + 1) * XCH],
                                      xT_d[:, kp * XCH:(kp + 1) * XCH])

                ct = dq.tile([KB, W2], BF)
                rd = dq.tile([KB, W2], BF)
                e3 = dq.tile([KB, W2], BF)
                f = rd if F_ALIAS else dq.tile([KB, W2], BF)

                # rd = relu(|d| - G), the tail excess, feeding the Exp pass.
                # DVE can't pair the bitwise sign-strip with an arith max in
                # one instruction, so this is two passes. On the first tile
                # run the h=0 block separately so Exp starts after 1 DMA.
                rd_slices = ([slice(0, O_SH), slice(O_SH, W2)]
                             if kp == 0 and HEAD_FINE else [slice(0, W2)])
                for rs in rd_slices:
                    nc.vector.tensor_scalar(rd[:, rs].bitcast(U16),
                                            wt[:, rs].bitcast(U16), 0x7FFF,
                                            None, Alu.bitwise_and)
                    nc.vector.tensor_scalar(rd[:, rs], rd[:, rs], g, g,
                                            Alu.max, Alu.subtract)
                # core = clamp(d, -G, G); Pool's merge columns first so its
                # Multiply isn't gated on the whole pass
                nc.vector.tensor_scalar(ct[:, :POOL_COLS], wt[:, :POOL_COLS],
                                        -g, g, Alu.max, Alu.min)
                nc.vector.tensor_scalar(ct[:, POOL_COLS:], wt[:, POOL_COLS:],
                                        -g, g, Alu.max, Alu.min)
                # e = exp(rd / scale) >= 1, then f = core * e (sign rides
                # on core; e > 0) — both in column slices so downstream
                # stages start before the full tile is done
                AW = W2 // ACT_HALVES
                for hh in range(ACT_HALVES):
                    hs = slice(hh * AW, (hh + 1) * AW)
                    nc.scalar.activation(e3[:, hs], rd[:, hs], Act.Exp,
                                         bias=b0c[:], scale=inv_s)
                # merge f = core * e; Pool (GPSIMD Multiply) takes the FIRST
                # POOL_COLS columns (their e3 lands first, so Pool starts
                # early), DVE the rest in TT_HALVES slices
                PW = POOL_COLS // POOL_HALVES if POOL_COLS else 0
                for hh in range(POOL_HALVES if POOL_COLS else 0):
                    hs = slice(hh * PW, (hh + 1) * PW)
                    nc.gpsimd.tensor_tensor(f[:, hs], ct[:, hs],
                                            e3[:, hs], Alu.mult)
                TW = DC // TT_HALVES
                for hh in range(TT_HALVES):
                    hs = slice(POOL_COLS + hh * TW, POOL_COLS + (hh + 1) * TW)
                    nc.vector.tensor_tensor(f[:, hs], ct[:, hs],
                                            e3[:, hs], Alu.mult)

                last = kp == NKB // NH - 1
                # mi-major on the final tile so psum mi=0 closes (and can
                # evict) while mi=1's matmuls still run
                if last:
                    order = [(h, mi) for mi in range(NMB) for h in range(NH)]
                else:
                    order = [(h, mi) for h in range(NH) for mi in range(NMB)]
                for h, mi in order:
                    kb = NH * kp + h
                    lhsT = xT_sb[:, kb * M + mi * MB:
                                 kb * M + (mi + 1) * MB]
                    for oc in range(NOC):
                        sl = slice(h * O_SH + oc * OC,
                                   h * O_SH + (oc + 1) * OC)
                        psl = slice(oc * OC, (oc + 1) * OC)
                        nc.tensor.matmul(psums[mi][:, psl], lhsT,
                                         f[:, sl], start=False,
                                         stop=last and h == NH - 1)

            # evict each PSUM in column halves, Act and DVE in parallel
            HO = O_SH // 2
            for mi in range(NMB):
                osb = misc.tile([MB, O_SH], ODT, name=f"osb{mi}",
                                tag=f"osb{mi}")
                nc.scalar.copy(osb[:, :HO], psums[mi][:, :HO])
                nc.vector.tensor_scalar(osb[:, HO:], psums[mi][:, HO:],
                                        0.0, None, Alu.add)
                nc.sync.dma_start(out_d[mi * MB:(mi + 1) * MB, :], osb[:])

    nc.compile()
    return nc


def _get_nc(inv_s, g):
    key = (round(inv_s, 12), round(g, 12))
    if key not in _CACHE:
        _CACHE[key] = _build(inv_s, g)
    return _CACHE[key]


def _prep_inputs(x, epsilon, gamma, scale, bias, weight_q):
    eps = float(np.asarray(epsilon).ravel()[0])
    gam = float(np.asarray(gamma).ravel()[0])
    sc = float(np.asarray(scale).ravel()[0])
    alpha = (gam - eps) / sc
    assert alpha > 0
    k_sign = eps / alpha
    g = gam / alpha
    inv_s = 1.0 / sc

    xr = np.asarray(x, dtype=np.float32).reshape(M, IN) * np.float32(alpha)
    xT = np.ascontiguousarray(xr.T)                       # [IN, M]
    xT_blocked = np.ascontiguousarray(
        xT.reshape(NKB, KB, M).transpose(1, 0, 2)
    ).reshape(KB, NKB * M).astype(bfloat16)

    # un-center the integer codes: d = q + K*sign(q), exact in bf16
    wq = np.asarray(weight_q, dtype=np.float32)
    wd = (wq + np.float32(k_sign) * np.sign(wq)).astype(bfloat16)
    bias_bf = np.asarray(bias, dtype=np.float32).astype(bfloat16)

    in_maps = []
    for c in range(N_CORES):
        wTc = np.ascontiguousarray(
            wd[c * O_SH:(c + 1) * O_SH, :].T)             # [IN, O_SH]
        in_maps.append({
            "wT": wTc,
            "xT": xT_blocked,
            "bias": bias_bf[c * O_SH:(c + 1) * O_SH].reshape(1, O_SH),
        })
    return (inv_s, g), in_maps


def _run(nc, in_maps, **kw):
    from concourse import bass_utils
    return bass_utils.run_bass_kernel_spmd(
        nc, in_maps, core_ids=list(range(N_CORES)), **kw)


def kernel(x, epsilon, gamma, scale, bias, weight_q):
    consts, in_maps = _prep_inputs(x, epsilon, gamma, scale, bias, weight_q)
    nc = _get_nc(*consts)
    res = _run(nc, in_maps)
    out = np.concatenate(
        [np.asarray(res.results[c]["out"]) for c in range(N_CORES)], axis=1)
    return np.ascontiguousarray(out.reshape(B, S, OUT)).astype(np.float32)


# revision 50
# speedup vs baseline: 1.9467x; 1.0070x over previous
"""Trainium2 Bass kernel for CustomCenterQuantizerLinear.

Computes out = x @ f(weight_q).T + bias over 8 NeuronCores, where f is the
piecewise dequantizer:
    y = q / scale
    f = sign(y) * (eps + |y|*(gam-eps))        for |y| <= 1
    f = sign(y) * gam * exp(|y| - 1)           for |y| >  1
    f = 0                                      for y == 0

Sharding: tensor-parallel column split of weight/bias over out_features
(1024 per core), x replicated.

Device math (exact, in alpha-units with alpha=(gam-eps)/scale, K=eps/alpha,
G=gam/alpha = K+scale): the host un-centers the integer codes,
    d = q + K*sign(q)            (0 -> 0; |d| = |q|+K in [K+1, K+127])
and the device evaluates the multiplicative form
    core = clamp(d, -G, G)                    # signed, = sgn*min(|q|+K, G)
    rd   = max(|d|, G) - G                    # = relu(|q| - scale)
    e    = exp(rd / scale)                    # = 1 in-range, e^(|y|-1) in tail
    f    = core * e                           # sign carried by the multiply
which matches f/alpha exactly on all integer codes (both branches agree at
the |q|=scale breakpoint because clamp and relu share it). alpha is folded
into x on the host.

Per 4-k-block tile [128, 4096] this costs two 4x-mode tensor_scalar passes
plus one 2x tensor_tensor on DVE, one Exp pass on Act, and a single
PSUM-accumulated matmul stream on PE; a slice of the relu pass runs on the
otherwise-idle Pool engine to keep DVE at the DMA roofline.
"""

import sys

sys.path.insert(0, "/opt/trn_rl_repo")

import numpy as np
from ml_dtypes import bfloat16

B, S, IN, OUT = 8, 32, 8192, 8192
N_CORES = 8
M = B * S                 # 256 tokens
O_SH = OUT // N_CORES     # 1024 out features per core
KB = 128                  # contraction block (PE partition dim)
NKB = IN // KB            # 64 k-blocks
MB = 128                  # token block (PSUM partition dim)
NMB = M // MB             # 2 token blocks
OC = 512                  # matmul free-dim chunk (one PSUM bank)
NOC = O_SH // OC          # 2 chunks
NH = 4                    # k-blocks per dequant tile
W2 = NH * O_SH            # dequant tile width (4096)
POOL_COLS = 2048          # columns of the merge offloaded to Pool (Multiply)
OUT_BF16 = True           # evict PSUM to bf16 (halves output DMA)
DIRECT_EVICT = False      # (unsupported: PSUM->HBM DMA rejected by bass)
ACT_HALVES = 4            # exp pass split factor (must divide W2)
TT_HALVES = 1             # DVE merge split factor
POOL_HALVES = 2           # pool merge split factor
HEAD_FINE = True          # fine-grained first tile (shorter pipeline fill)
POOL_LAST = 1536          # pool merge columns on the final tile (tail)
WP_BUFS = 3               # weight-tile double buffering depth
DQ_BUFS = 4               # dequant-tile pool depth
F_ALIAS = False           # write f in-place over rd

_CACHE = {}


def _build(inv_s, g):
    import concourse.bass as bass
    import concourse.bacc as bacc
    import concourse.mybir as mybir
    import concourse.tile as tile

    BF = mybir.dt.bfloat16
    F32 = mybir.dt.float32
    U16 = mybir.dt.uint16
    Alu = mybir.AluOpType
    Act = mybir.ActivationFunctionType
    gbits = int(np.asarray(g, dtype=bfloat16).view(np.uint16))

    nc = bacc.Bacc("TRN2", target_bir_lowering=False, debug=False,
                   num_devices=N_CORES)
    ODT = BF if OUT_BF16 else F32
    wT_d = nc.dram_tensor("wT", [IN, O_SH], BF, kind="ExternalInput").ap()
    xT_d = nc.dram_tensor("xT", [KB, NKB * M], BF, kind="ExternalInput").ap()
    bias_d = nc.dram_tensor("bias", [1, O_SH], BF, kind="ExternalInput").ap()
    out_d = nc.dram_tensor("out", [M, O_SH], ODT, kind="ExternalOutput").ap()

    DC = W2 - POOL_COLS       # columns of the relu pass kept on DVE

    with tile.TileContext(nc) as tc:
        with (
            tc.tile_pool(name="misc", bufs=1) as misc,
            tc.tile_pool(name="wp", bufs=WP_BUFS) as wp,
            tc.tile_pool(name="dq", bufs=DQ_BUFS) as dq,
            tc.tile_pool(name="psum", bufs=1, space=bass.MemorySpace.PSUM) as pp,
        ):
            xT_sb = misc.tile([KB, NKB * M], BF)
            bias_sb = misc.tile([1, O_SH], BF)
            ones_sb = misc.tile([1, MB], BF)
            nc.gpsimd.dma_start(bias_sb[:], bias_d[:])
            nc.vector.memset(ones_sb[:], 1.0)
            b0c = misc.tile([128, 1], F32)
            nc.vector.memset(b0c[:], 0.0)

            psums = [pp.tile([MB, O_SH], F32, name=f"ps{mi}", tag=f"ps{mi}")
                     for mi in range(NMB)]

            # seed the accumulators with the bias so the tail has no extra
            # matmul round: psum = ones^T @ bias, start=True
            for mi in range(NMB):
                for oc in range(NOC):
                    sl = slice(oc * OC, (oc + 1) * OC)
                    nc.tensor.matmul(psums[mi][:, sl], ones_sb[:],
                                     bias_sb[:, sl], start=True, stop=False)

            XCH = NH * M         # x columns consumed per kp iteration
            for kp in range(NKB // NH):
                wt = wp.tile([KB, W2], BF)
                if kp == 0 and HEAD_FINE:
                    # interleave the first x half between the first weight
                    # blocks so the h=0 matmuls aren't gated on 1MB+x
                    nc.sync.dma_start(wt[:, :O_SH], wT_d[:KB, :])
                    nc.sync.dma_start(xT_sb[:, :XCH // 2],
                                      xT_d[:, :XCH // 2])
                    for h in range(1, NH):
                        nc.sync.dma_start(wt[:, h * O_SH:(h + 1) * O_SH],
                                          wT_d[h * KB:(h + 1) * KB, :])
                    nc.sync.dma_start(xT_sb[:, XCH // 2:XCH],
                                      xT_d[:, XCH // 2:XCH])
                else:
                    for h in range(NH):
                        kb = NH * kp + h
                        nc.sync.dma_start(
                            wt[:, h * O_SH:(h + 1) * O_SH],
                            wT_d[kb * KB:(kb + 1) * KB, :])
                    # x arrives just-in-time, one kp-slice behind the
                    # weights, so the head isn't gated on the full 4.2MB
                    nc.sync.dma_start(xT_sb[:, kp * XCH:(kp + 1) * XCH],
                                      xT_d[:, kp * XCH:(kp + 1) * XCH])

                ct = dq.tile([KB, W2], BF)
                rd = dq.tile([KB, W2], BF)
                e3 = dq.tile([KB, W2], BF)
                f = rd if F_ALIAS else dq.tile([KB, W2], BF)

                last = kp == NKB // NH - 1
                # rd = relu(|d| - G), the tail excess, feeding the Exp pass.
                # DVE can't pair the bitwise sign-strip with an arith max in
                # one instruction, so this is two passes. On the first tile
                # run the h=0 block separately so Exp starts after 1 DMA.
                rd_slices = ([slice(0, O_SH), slice(O_SH, W2)]
                             if kp == 0 and HEAD_FINE else [slice(0, W2)])
                for rs in rd_slices:
                    nc.vector.tensor_scalar(rd[:, rs].bitcast(U16),
                                            wt[:, rs].bitcast(U16), 0x7FFF,
                                            None, Alu.bitwise_and)
                    nc.vector.tensor_scalar(rd[:, rs], rd[:, rs], g, g,
                                            Alu.max, Alu.subtract)
                # core = clamp(d, -G, G); Pool's merge columns first so its
                # Multiply isn't gated on the whole pass
                nc.vector.tensor_scalar(ct[:, :POOL_COLS], wt[:, :POOL_COLS],
                                        -g, g, Alu.max, Alu.min)
                nc.vector.tensor_scalar(ct[:, POOL_COLS:], wt[:, POOL_COLS:],
                                        -g, g, Alu.max, Alu.min)
                # e = exp(rd / scale) >= 1, then f = core * e (sign rides
                # on core; e > 0) — both in column slices so downstream
                # stages start before the full tile is done
                AW = W2 // ACT_HALVES
                for hh in range(ACT_HALVES):
                    hs = slice(hh * AW, (hh + 1) * AW)
                    nc.scalar.activation(e3[:, hs], rd[:, hs], Act.Exp,
                                         bias=b0c[:], scale=inv_s)
                # merge f = core * e; Pool (GPSIMD Multiply) takes the FIRST
                # POOL_COLS columns (their e3 lands first, so Pool starts
                # early), DVE the rest in TT_HALVES slices
                pc = POOL_LAST if last else POOL_COLS
                dc = W2 - pc
                PW = pc // POOL_HALVES if pc else 0
                for hh in range(POOL_HALVES if pc else 0):
                    hs = slice(hh * PW, (hh + 1) * PW)
                    nc.gpsimd.tensor_tensor(f[:, hs], ct[:, hs],
                                            e3[:, hs], Alu.mult)
                TW = dc // TT_HALVES
                for hh in range(TT_HALVES):
                    hs = slice(pc + hh * TW, pc + (hh + 1) * TW)
                    nc.vector.tensor_tensor(f[:, hs], ct[:, hs],
                                            e3[:, hs], Alu.mult)

                # mi-major on the final tile so psum mi=0 closes (and can
                # evict) while mi=1's matmuls still run
                if last:
                    order = [(h, mi) for mi in range(NMB) for h in range(NH)]
                else:
                    order = [(h, mi) for h in range(NH) for mi in range(NMB)]
                for h, mi in order:
                    kb = NH * kp + h
                    lhsT = xT_sb[:, kb * M + mi * MB:
                                 kb * M + (mi + 1) * MB]
                    for oc in range(NOC):
                        sl = slice(h * O_SH + oc * OC,
                                   h * O_SH + (oc + 1) * OC)
                        psl = slice(oc * OC, (oc + 1) * OC)
                        nc.tensor.matmul(psums[mi][:, psl], lhsT,
                                         f[:, sl], start=False,
                                         stop=last and h == NH - 1)

            # evict each PSUM in column halves, Act and DVE in parallel,
            # each half DMA'd out as soon as it lands in SBUF
            HO = O_SH // 2
            for mi in range(NMB):
                osb = misc.tile([MB, O_SH], ODT, name=f"osb{mi}",
                                tag=f"osb{mi}")
                nc.scalar.copy(osb[:, :HO], psums[mi][:, :HO])
                nc.vector.tensor_scalar(osb[:, HO:], psums[mi][:, HO:],
                                        0.0, None, Alu.add)
                nc.sync.dma_start(out_d[mi * MB:(mi + 1) * MB, :], osb[:])

    nc.compile()
    return nc


def _get_nc(inv_s, g):
    key = (round(inv_s, 12), round(g, 12))
    if key not in _CACHE:
        _CACHE[key] = _build(inv_s, g)
    return _CACHE[key]


def _prep_inputs(x, epsilon, gamma, scale, bias, weight_q):
    eps = float(np.asarray(epsilon).ravel()[0])
    gam = float(np.asarray(gamma).ravel()[0])
    sc = float(np.asarray(scale).ravel()[0])
    alpha = (gam - eps) / sc
    assert alpha > 0
    k_sign = eps / alpha
    g = gam / alpha
    inv_s = 1.0 / sc

    xr = np.asarray(x, dtype=np.float32).reshape(M, IN) * np.float32(alpha)
    xT = np.ascontiguousarray(xr.T)                       # [IN, M]
    xT_blocked = np.ascontiguousarray(
        xT.reshape(NKB, KB, M).transpose(1, 0, 2)
    ).reshape(KB, NKB * M).astype(bfloat16)

    # un-center the integer codes: d = q + K*sign(q), exact in bf16
    wq = np.asarray(weight_q, dtype=np.float32)
    wd = (wq + np.float32(k_sign) * np.sign(wq)).astype(bfloat16)
    bias_bf = np.asarray(bias, dtype=np.float32).astype(bfloat16)

    in_maps = []
    for c in range(N_CORES):
        wTc = np.ascontiguousarray(
            wd[c * O_SH:(c + 1) * O_SH, :].T)             # [IN, O_SH]
        in_maps.append({
            "wT": wTc,
            "xT": xT_blocked,
            "bias": bias_bf[c * O_SH:(c + 1) * O_SH].reshape(1, O_SH),
        })
    return (inv_s, g), in_maps


def _run(nc, in_maps, **kw):
    from concourse import bass_utils
    return bass_utils.run_bass_kernel_spmd(
        nc, in_maps, core_ids=list(range(N_CORES)), **kw)


def kernel(x, epsilon, gamma, scale, bias, weight_q):
    consts, in_maps = _prep_inputs(x, epsilon, gamma, scale, bias, weight_q)
    nc = _get_nc(*consts)
    res = _run(nc, in_maps)
    out = np.concatenate(
        [np.asarray(res.results[c]["out"]) for c in range(N_CORES)], axis=1)
    return np.ascontiguousarray(out.reshape(B, S, OUT)).astype(np.float32)


# revision 61
# speedup vs baseline: 1.9728x; 1.0134x over previous
"""Trainium2 Bass kernel for CustomCenterQuantizerLinear.

Computes out = x @ f(weight_q).T + bias over 8 NeuronCores, where f is the
piecewise dequantizer:
    y = q / scale
    f = sign(y) * (eps + |y|*(gam-eps))        for |y| <= 1
    f = sign(y) * gam * exp(|y| - 1)           for |y| >  1
    f = 0                                      for y == 0

Sharding: tensor-parallel column split of weight/bias over out_features
(1024 per core), x replicated.

Device math (exact, in alpha-units with alpha=(gam-eps)/scale, K=eps/alpha,
G=gam/alpha = K+scale): the host un-centers the integer codes,
    d = q + K*sign(q)            (0 -> 0; |d| = |q|+K in [K+1, K+127])
and the device evaluates the multiplicative form
    core = clamp(d, -G, G)                    # signed, = sgn*min(|q|+K, G)
    rd   = max(|d|, G) - G                    # = relu(|q| - scale)
    e    = exp(rd / scale)                    # = 1 in-range, e^(|y|-1) in tail
    f    = core * e                           # sign carried by the multiply
which matches f/alpha exactly on all integer codes (both branches agree at
the |q|=scale breakpoint because clamp and relu share it). alpha is folded
into x on the host.

Per 4-k-block tile [128, 4096] this costs two 4x-mode tensor_scalar passes
plus one 2x tensor_tensor on DVE, one Exp pass on Act, and a single
PSUM-accumulated matmul stream on PE; a slice of the relu pass runs on the
otherwise-idle Pool engine to keep DVE at the DMA roofline.
"""

import sys

sys.path.insert(0, "/opt/trn_rl_repo")

import numpy as np
from ml_dtypes import bfloat16

B, S, IN, OUT = 8, 32, 8192, 8192
N_CORES = 8
M = B * S                 # 256 tokens
O_SH = OUT // N_CORES     # 1024 out features per core
KB = 128                  # contraction block (PE partition dim)
NKB = IN // KB            # 64 k-blocks
MB = 128                  # token block (PSUM partition dim)
NMB = M // MB             # 2 token blocks
OC = 512                  # matmul free-dim chunk (one PSUM bank)
NOC = O_SH // OC          # 2 chunks
NH = 4                    # k-blocks per dequant tile
W2 = NH * O_SH            # dequant tile width (4096)
POOL_COLS = 2048          # columns of the merge offloaded to Pool (Multiply)
OUT_BF16 = True           # evict PSUM to bf16 (halves output DMA)
ACT_HALVES = 4            # exp pass split factor (must divide W2)
TT_HALVES = 1             # DVE merge split factor
POOL_HALVES = 2           # pool merge split factor
HEAD_FINE = False         # fine-grained first tile (hurts with merged DMAs)
POOL_LAST = 1536          # pool merge columns on the final tile (tail)
TAIL_FINE = True          # final tile: per-1024-block merges
POOL_LAST_H = 1           # with TAIL_FINE: leading 1024-blocks on Pool
W_DMAS = 2                # weight DMAs per tile (contiguous column splits)
WP_BUFS = 3               # weight-tile double buffering depth
DQ_BUFS = 4               # dequant-tile pool depth
F_ALIAS = False           # write f in-place over rd

_CACHE = {}


def _build(inv_s, g):
    import concourse.bass as bass
    import concourse.bacc as bacc
    import concourse.mybir as mybir
    import concourse.tile as tile

    BF = mybir.dt.bfloat16
    F32 = mybir.dt.float32
    U16 = mybir.dt.uint16
    Alu = mybir.AluOpType
    Act = mybir.ActivationFunctionType

    nc = bacc.Bacc("TRN2", target_bir_lowering=False, debug=False,
                   num_devices=N_CORES)
    ODT = BF if OUT_BF16 else F32
    # weights arrive pre-tiled on the host: row-block kp holds the NH
    # k-blocks of one dequant tile side by side, so each tile is a single
    # contiguous [128 x 8KB-row] DMA
    wT_d = nc.dram_tensor("wT", [NKB // NH * KB, W2], BF,
                          kind="ExternalInput").ap()
    xT_d = nc.dram_tensor("xT", [KB, NKB * M], BF, kind="ExternalInput").ap()
    bias_d = nc.dram_tensor("bias", [1, O_SH], BF, kind="ExternalInput").ap()
    out_d = nc.dram_tensor("out", [M, O_SH], ODT, kind="ExternalOutput").ap()

    with tile.TileContext(nc) as tc:
        with (
            tc.tile_pool(name="misc", bufs=1) as misc,
            tc.tile_pool(name="wp", bufs=WP_BUFS) as wp,
            tc.tile_pool(name="dq", bufs=DQ_BUFS) as dq,
            tc.tile_pool(name="psum", bufs=1, space=bass.MemorySpace.PSUM) as pp,
        ):
            xT_sb = misc.tile([KB, NKB * M], BF)
            bias_sb = misc.tile([1, O_SH], BF)
            ones_sb = misc.tile([1, MB], BF)
            nc.gpsimd.dma_start(bias_sb[:], bias_d[:])
            nc.vector.memset(ones_sb[:], 1.0)
            b0c = misc.tile([128, 1], F32)
            nc.vector.memset(b0c[:], 0.0)

            psums = [pp.tile([MB, O_SH], F32, name=f"ps{mi}", tag=f"ps{mi}")
                     for mi in range(NMB)]

            # seed the accumulators with the bias so the tail has no extra
            # matmul round: psum = ones^T @ bias, start=True
            for mi in range(NMB):
                for oc in range(NOC):
                    sl = slice(oc * OC, (oc + 1) * OC)
                    nc.tensor.matmul(psums[mi][:, sl], ones_sb[:],
                                     bias_sb[:, sl], start=True, stop=False)

            XCH = NH * M         # x columns consumed per kp iteration
            for kp in range(NKB // NH):
                wt = wp.tile([KB, W2], BF)
                rows = slice(kp * KB, (kp + 1) * KB)
                if kp == 0 and HEAD_FINE:
                    # interleave the first x half between the first weight
                    # blocks so the h=0 matmuls aren't gated on 1MB+x
                    nc.sync.dma_start(wt[:, :O_SH], wT_d[rows, :O_SH])
                    nc.sync.dma_start(xT_sb[:, :XCH // 2],
                                      xT_d[:, :XCH // 2])
                    nc.sync.dma_start(wt[:, O_SH:], wT_d[rows, O_SH:])
                    nc.sync.dma_start(xT_sb[:, XCH // 2:XCH],
                                      xT_d[:, XCH // 2:XCH])
                else:
                    for ws in range(W_DMAS):
                        WS = W2 // W_DMAS
                        nc.sync.dma_start(wt[:, ws * WS:(ws + 1) * WS],
                                          wT_d[rows, ws * WS:(ws + 1) * WS])
                    # x arrives just-in-time, one kp-slice behind the
                    # weights, so the head isn't gated on the full 4.2MB
                    nc.sync.dma_start(xT_sb[:, kp * XCH:(kp + 1) * XCH],
                                      xT_d[:, kp * XCH:(kp + 1) * XCH])

                ct = dq.tile([KB, W2], BF)
                rd = dq.tile([KB, W2], BF)
                e3 = dq.tile([KB, W2], BF)
                f = rd if F_ALIAS else dq.tile([KB, W2], BF)

                last = kp == NKB // NH - 1
                # rd = relu(|d| - G), the tail excess, feeding the Exp pass.
                # DVE can't pair the bitwise sign-strip with an arith max in
                # one instruction, so this is two passes. On the first tile
                # run the h=0 block separately so Exp starts after 1 DMA.
                rd_slices = ([slice(0, O_SH), slice(O_SH, W2)]
                             if kp == 0 and HEAD_FINE else [slice(0, W2)])
                for rs in rd_slices:
                    nc.vector.tensor_scalar(rd[:, rs].bitcast(U16),
                                            wt[:, rs].bitcast(U16), 0x7FFF,
                                            None, Alu.bitwise_and)
                    nc.vector.tensor_scalar(rd[:, rs], rd[:, rs], g, g,
                                            Alu.max, Alu.subtract)
                # core = clamp(d, -G, G); Pool's merge columns first so its
                # Multiply isn't gated on the whole pass (first tile: the
                # h=0 block alone so its merge fires right away)
                ct_pts = ([O_SH, POOL_COLS] if kp == 0 and HEAD_FINE
                          else [POOL_COLS])
                for lo, hi in zip([0] + ct_pts, ct_pts + [W2]):
                    nc.vector.tensor_scalar(ct[:, lo:hi], wt[:, lo:hi],
                                            -g, g, Alu.max, Alu.min)
                # e = exp(rd / scale) >= 1, then f = core * e (sign rides
                # on core; e > 0) — both in column slices so downstream
                # stages start before the full tile is done
                AW = W2 // ACT_HALVES
                for hh in range(ACT_HALVES):
                    hs = slice(hh * AW, (hh + 1) * AW)
                    nc.scalar.activation(e3[:, hs], rd[:, hs], Act.Exp,
                                         bias=b0c[:], scale=inv_s)
                # merge f = core * e; Pool (GPSIMD Multiply) takes the FIRST
                # POOL_COLS columns (their e3 lands first, so Pool starts
                # early), DVE the rest in TT_HALVES slices
                fine_tail = kp >= NKB // NH - TAIL_FINE
                if fine_tail:
                    # final tile: merge per 1024-block right behind each Exp
                    # quarter so the closing matmuls drain early
                    for h in range(NH):
                        hs = slice(h * O_SH, (h + 1) * O_SH)
                        eng = nc.gpsimd if h < POOL_LAST_H else nc.vector
                        eng.tensor_tensor(f[:, hs], ct[:, hs],
                                          e3[:, hs], Alu.mult)
                else:
                    pc = POOL_LAST if last else POOL_COLS
                    dc = W2 - pc
                    PW = pc // POOL_HALVES if pc else 0
                    for hh in range(POOL_HALVES if pc else 0):
                        hs = slice(hh * PW, (hh + 1) * PW)
                        nc.gpsimd.tensor_tensor(f[:, hs], ct[:, hs],
                                                e3[:, hs], Alu.mult)
                    TW = dc // TT_HALVES
                    for hh in range(TT_HALVES):
                        hs = slice(pc + hh * TW, pc + (hh + 1) * TW)
                        nc.vector.tensor_tensor(f[:, hs], ct[:, hs],
                                                e3[:, hs], Alu.mult)

                # mi-major on the final tile so psum mi=0 closes (and can
                # evict) while mi=1's matmuls still run
                if last:
                    order = [(h, mi) for mi in range(NMB) for h in range(NH)]
                else:
                    order = [(h, mi) for h in range(NH) for mi in range(NMB)]
                for h, mi in order:
                    kb = NH * kp + h
                    lhsT = xT_sb[:, kb * M + mi * MB:
                                 kb * M + (mi + 1) * MB]
                    for oc in range(NOC):
                        sl = slice(h * O_SH + oc * OC,
                                   h * O_SH + (oc + 1) * OC)
                        psl = slice(oc * OC, (oc + 1) * OC)
                        nc.tensor.matmul(psums[mi][:, psl], lhsT,
                                         f[:, sl], start=False,
                                         stop=last and h == NH - 1)

            # evict each PSUM in column halves, Act and DVE in parallel,
            # each half DMA'd out as soon as it lands in SBUF
            HO = O_SH // 2
            for mi in range(NMB):
                osb = misc.tile([MB, O_SH], ODT, name=f"osb{mi}",
                                tag=f"osb{mi}")
                nc.scalar.copy(osb[:, :HO], psums[mi][:, :HO])
                nc.vector.tensor_scalar(osb[:, HO:], psums[mi][:, HO:],
                                        0.0, None, Alu.add)
                nc.sync.dma_start(out_d[mi * MB:(mi + 1) * MB, :], osb[:])

    nc.compile()
    return nc


def _get_nc(inv_s, g):
    key = (round(inv_s, 12), round(g, 12))
    if key not in _CACHE:
        _CACHE[key] = _build(inv_s, g)
    return _CACHE[key]


def _prep_inputs(x, epsilon, gamma, scale, bias, weight_q):
    eps = float(np.asarray(epsilon).ravel()[0])
    gam = float(np.asarray(gamma).ravel()[0])
    sc = float(np.asarray(scale).ravel()[0])
    alpha = (gam - eps) / sc
    assert alpha > 0
    k_sign = eps / alpha
    g = gam / alpha
    inv_s = 1.0 / sc

    xr = np.asarray(x, dtype=np.float32).reshape(M, IN) * np.float32(alpha)
    xT = np.ascontiguousarray(xr.T)                       # [IN, M]
    xT_blocked = np.ascontiguousarray(
        xT.reshape(NKB, KB, M).transpose(1, 0, 2)
    ).reshape(KB, NKB * M).astype(bfloat16)

    # un-center the integer codes: d = q + K*sign(q), exact in bf16
    wq = np.asarray(weight_q, dtype=np.float32)
    wd = (wq + np.float32(k_sign) * np.sign(wq)).astype(bfloat16)
    bias_bf = np.asarray(bias, dtype=np.float32).astype(bfloat16)

    in_maps = []
    for c in range(N_CORES):
        wTc = np.ascontiguousarray(
            wd[c * O_SH:(c + 1) * O_SH, :].T)             # [IN, O_SH]
        # pre-tile: row-block kp = its NH k-blocks side by side, so the
        # device loads each dequant tile as one contiguous DMA
        wTt = np.ascontiguousarray(
            wTc.reshape(NKB // NH, NH, KB, O_SH).transpose(0, 2, 1, 3)
        ).reshape(NKB // NH * KB, NH * O_SH)
        in_maps.append({
            "wT": wTt,
            "xT": xT_blocked,
            "bias": bias_bf[c * O_SH:(c + 1) * O_SH].reshape(1, O_SH),
        })
    return (inv_s, g), in_maps


def _run(nc, in_maps, **kw):
    from concourse import bass_utils
    return bass_utils.run_bass_kernel_spmd(
        nc, in_maps, core_ids=list(range(N_CORES)), **kw)


def kernel(x, epsilon, gamma, scale, bias, weight_q):
    consts, in_maps = _prep_inputs(x, epsilon, gamma, scale, bias, weight_q)
    nc = _get_nc(*consts)
    res = _run(nc, in_maps)
    out = np.concatenate(
        [np.asarray(res.results[c]["out"]) for c in range(N_CORES)], axis=1)
    return np.ascontiguousarray(out.reshape(B, S, OUT)).astype(np.float32)


# revision 63
# speedup vs baseline: 1.9733x; 1.0003x over previous
"""Trainium2 Bass kernel for CustomCenterQuantizerLinear.

Computes out = x @ f(weight_q).T + bias over 8 NeuronCores, where f is the
piecewise dequantizer:
    y = q / scale
    f = sign(y) * (eps + |y|*(gam-eps))        for |y| <= 1
    f = sign(y) * gam * exp(|y| - 1)           for |y| >  1
    f = 0                                      for y == 0

Sharding: tensor-parallel column split of weight/bias over out_features
(1024 per core), x replicated.

Device math (exact, in alpha-units with alpha=(gam-eps)/scale, K=eps/alpha,
G=gam/alpha = K+scale): the host un-centers the integer codes,
    d = q + K*sign(q)            (0 -> 0; |d| = |q|+K in [K+1, K+127])
and the device evaluates the multiplicative form
    core = clamp(d, -G, G)                    # signed, = sgn*min(|q|+K, G)
    rd   = max(|d|, G) - G                    # = relu(|q| - scale)
    e    = exp(rd / scale)                    # = 1 in-range, e^(|y|-1) in tail
    f    = core * e                           # sign carried by the multiply
which matches f/alpha exactly on all integer codes (both branches agree at
the |q|=scale breakpoint because clamp and relu share it). alpha is folded
into x on the host.

Per 4-k-block tile [128, 4096] this costs two 4x-mode tensor_scalar passes
plus one 2x tensor_tensor on DVE, one Exp pass on Act, and a single
PSUM-accumulated matmul stream on PE; a slice of the relu pass runs on the
otherwise-idle Pool engine to keep DVE at the DMA roofline.
"""

import sys

sys.path.insert(0, "/opt/trn_rl_repo")

import numpy as np
from ml_dtypes import bfloat16

B, S, IN, OUT = 8, 32, 8192, 8192
N_CORES = 8
M = B * S                 # 256 tokens
O_SH = OUT // N_CORES     # 1024 out features per core
KB = 128                  # contraction block (PE partition dim)
NKB = IN // KB            # 64 k-blocks
MB = 128                  # token block (PSUM partition dim)
NMB = M // MB             # 2 token blocks
OC = 512                  # matmul free-dim chunk (one PSUM bank)
NOC = O_SH // OC          # 2 chunks
NH = 4                    # k-blocks per dequant tile
W2 = NH * O_SH            # dequant tile width (4096)
POOL_COLS = 2048          # columns of the merge offloaded to Pool (Multiply)
OUT_BF16 = True           # evict PSUM to bf16 (halves output DMA)
ACT_HALVES = 4            # exp pass split factor (must divide W2)
TT_HALVES = 1             # DVE merge split factor
HEAD_FINE = False         # fine-grained first tile (hurts with merged DMAs)
POOL_LAST = 1536          # pool merge columns on the final tile (tail)
TAIL_FINE = True          # final tile: per-1024-block merges
POOL_LAST_H = 1           # with TAIL_FINE: leading 1024-cols on Pool
TAIL_BLK = 1024           # final-tile merge block width
POOL_HALVES = 4           # pool merge split factor (override below)
W_DMAS = 2                # weight DMAs per tile (contiguous column splits)
WP_BUFS = 3               # weight-tile double buffering depth
DQ_BUFS = 4               # dequant-tile pool depth
F_ALIAS = False           # write f in-place over rd

_CACHE = {}


def _build(inv_s, g):
    import concourse.bass as bass
    import concourse.bacc as bacc
    import concourse.mybir as mybir
    import concourse.tile as tile

    BF = mybir.dt.bfloat16
    F32 = mybir.dt.float32
    U16 = mybir.dt.uint16
    Alu = mybir.AluOpType
    Act = mybir.ActivationFunctionType

    nc = bacc.Bacc("TRN2", target_bir_lowering=False, debug=False,
                   num_devices=N_CORES)
    ODT = BF if OUT_BF16 else F32
    # weights arrive pre-tiled on the host: row-block kp holds the NH
    # k-blocks of one dequant tile side by side, so each tile is a single
    # contiguous [128 x 8KB-row] DMA
    wT_d = nc.dram_tensor("wT", [NKB // NH * KB, W2], BF,
                          kind="ExternalInput").ap()
    xT_d = nc.dram_tensor("xT", [KB, NKB * M], BF, kind="ExternalInput").ap()
    bias_d = nc.dram_tensor("bias", [1, O_SH], BF, kind="ExternalInput").ap()
    out_d = nc.dram_tensor("out", [M, O_SH], ODT, kind="ExternalOutput").ap()

    with tile.TileContext(nc) as tc:
        with (
            tc.tile_pool(name="misc", bufs=1) as misc,
            tc.tile_pool(name="wp", bufs=WP_BUFS) as wp,
            tc.tile_pool(name="dq", bufs=DQ_BUFS) as dq,
            tc.tile_pool(name="psum", bufs=1, space=bass.MemorySpace.PSUM) as pp,
        ):
            xT_sb = misc.tile([KB, NKB * M], BF)
            bias_sb = misc.tile([1, O_SH], BF)
            ones_sb = misc.tile([1, MB], BF)
            nc.gpsimd.dma_start(bias_sb[:], bias_d[:])
            nc.vector.memset(ones_sb[:], 1.0)
            b0c = misc.tile([128, 1], F32)
            nc.vector.memset(b0c[:], 0.0)

            psums = [pp.tile([MB, O_SH], F32, name=f"ps{mi}", tag=f"ps{mi}")
                     for mi in range(NMB)]

            # seed the accumulators with the bias so the tail has no extra
            # matmul round: psum = ones^T @ bias, start=True
            for mi in range(NMB):
                for oc in range(NOC):
                    sl = slice(oc * OC, (oc + 1) * OC)
                    nc.tensor.matmul(psums[mi][:, sl], ones_sb[:],
                                     bias_sb[:, sl], start=True, stop=False)

            XCH = NH * M         # x columns consumed per kp iteration
            for kp in range(NKB // NH):
                wt = wp.tile([KB, W2], BF)
                rows = slice(kp * KB, (kp + 1) * KB)
                if kp == 0 and HEAD_FINE:
                    # interleave the first x half between the first weight
                    # blocks so the h=0 matmuls aren't gated on 1MB+x
                    nc.sync.dma_start(wt[:, :O_SH], wT_d[rows, :O_SH])
                    nc.sync.dma_start(xT_sb[:, :XCH // 2],
                                      xT_d[:, :XCH // 2])
                    nc.sync.dma_start(wt[:, O_SH:], wT_d[rows, O_SH:])
                    nc.sync.dma_start(xT_sb[:, XCH // 2:XCH],
                                      xT_d[:, XCH // 2:XCH])
                else:
                    for ws in range(W_DMAS):
                        WS = W2 // W_DMAS
                        nc.sync.dma_start(wt[:, ws * WS:(ws + 1) * WS],
                                          wT_d[rows, ws * WS:(ws + 1) * WS])
                    # x arrives just-in-time, one kp-slice behind the
                    # weights, so the head isn't gated on the full 4.2MB
                    nc.sync.dma_start(xT_sb[:, kp * XCH:(kp + 1) * XCH],
                                      xT_d[:, kp * XCH:(kp + 1) * XCH])

                ct = dq.tile([KB, W2], BF)
                rd = dq.tile([KB, W2], BF)
                e3 = dq.tile([KB, W2], BF)
                f = rd if F_ALIAS else dq.tile([KB, W2], BF)

                last = kp == NKB // NH - 1
                # rd = relu(|d| - G), the tail excess, feeding the Exp pass.
                # DVE can't pair the bitwise sign-strip with an arith max in
                # one instruction, so this is two passes. On the first tile
                # run the h=0 block separately so Exp starts after 1 DMA.
                rd_slices = ([slice(0, O_SH), slice(O_SH, W2)]
                             if kp == 0 and HEAD_FINE else [slice(0, W2)])
                for rs in rd_slices:
                    nc.vector.tensor_scalar(rd[:, rs].bitcast(U16),
                                            wt[:, rs].bitcast(U16), 0x7FFF,
                                            None, Alu.bitwise_and)
                    nc.vector.tensor_scalar(rd[:, rs], rd[:, rs], g, g,
                                            Alu.max, Alu.subtract)
                # core = clamp(d, -G, G); Pool's merge columns first so its
                # Multiply isn't gated on the whole pass (first tile: the
                # h=0 block alone so its merge fires right away)
                ct_pts = ([O_SH, POOL_COLS] if kp == 0 and HEAD_FINE
                          else [POOL_COLS])
                for lo, hi in zip([0] + ct_pts, ct_pts + [W2]):
                    nc.vector.tensor_scalar(ct[:, lo:hi], wt[:, lo:hi],
                                            -g, g, Alu.max, Alu.min)
                # e = exp(rd / scale) >= 1, then f = core * e (sign rides
                # on core; e > 0) — both in column slices so downstream
                # stages start before the full tile is done
                AW = W2 // ACT_HALVES
                for hh in range(ACT_HALVES):
                    hs = slice(hh * AW, (hh + 1) * AW)
                    nc.scalar.activation(e3[:, hs], rd[:, hs], Act.Exp,
                                         bias=b0c[:], scale=inv_s)
                # merge f = core * e; Pool (GPSIMD Multiply) takes the FIRST
                # POOL_COLS columns (their e3 lands first, so Pool starts
                # early), DVE the rest in TT_HALVES slices
                fine_tail = kp >= NKB // NH - TAIL_FINE
                if fine_tail:
                    # final tile: merge per TAIL_BLK-block right behind each
                    # Exp quarter so the closing matmuls drain early
                    nblk = W2 // TAIL_BLK
                    pool_blk = POOL_LAST_H * O_SH // TAIL_BLK
                    for h in range(nblk):
                        hs = slice(h * TAIL_BLK, (h + 1) * TAIL_BLK)
                        eng = nc.gpsimd if h < pool_blk else nc.vector
                        eng.tensor_tensor(f[:, hs], ct[:, hs],
                                          e3[:, hs], Alu.mult)
                else:
                    pc = POOL_LAST if last else POOL_COLS
                    dc = W2 - pc
                    PW = pc // POOL_HALVES if pc else 0
                    for hh in range(POOL_HALVES if pc else 0):
                        hs = slice(hh * PW, (hh + 1) * PW)
                        nc.gpsimd.tensor_tensor(f[:, hs], ct[:, hs],
                                                e3[:, hs], Alu.mult)
                    TW = dc // TT_HALVES
                    for hh in range(TT_HALVES):
                        hs = slice(pc + hh * TW, pc + (hh + 1) * TW)
                        nc.vector.tensor_tensor(f[:, hs], ct[:, hs],
                                                e3[:, hs], Alu.mult)

                # mi-major on the final tile so psum mi=0 closes (and can
                # evict) while mi=1's matmuls still run
                if last:
                    order = [(h, mi) for mi in range(NMB) for h in range(NH)]
                else:
                    order = [(h, mi) for h in range(NH) for mi in range(NMB)]
                for h, mi in order:
                    kb = NH * kp + h
                    lhsT = xT_sb[:, kb * M + mi * MB:
                                 kb * M + (mi + 1) * MB]
                    for oc in range(NOC):
                        sl = slice(h * O_SH + oc * OC,
                                   h * O_SH + (oc + 1) * OC)
                        psl = slice(oc * OC, (oc + 1) * OC)
                        nc.tensor.matmul(psums[mi][:, psl], lhsT,
                                         f[:, sl], start=False,
                                         stop=last and h == NH - 1)

            # evict each PSUM in column halves, Act and DVE in parallel,
            # each half DMA'd out as soon as it lands in SBUF
            HO = O_SH // 2
            for mi in range(NMB):
                osb = misc.tile([MB, O_SH], ODT, name=f"osb{mi}",
                                tag=f"osb{mi}")
                nc.scalar.copy(osb[:, :HO], psums[mi][:, :HO])
                nc.vector.tensor_scalar(osb[:, HO:], psums[mi][:, HO:],
                                        0.0, None, Alu.add)
                nc.sync.dma_start(out_d[mi * MB:(mi + 1) * MB, :], osb[:])

    nc.compile()
    return nc


def _get_nc(inv_s, g):
    key = (round(inv_s, 12), round(g, 12))
    if key not in _CACHE:
        _CACHE[key] = _build(inv_s, g)
    return _CACHE[key]


def _prep_inputs(x, epsilon, gamma, scale, bias, weight_q):
    eps = float(np.asarray(epsilon).ravel()[0])
    gam = float(np.asarray(gamma).ravel()[0])
    sc = float(np.asarray(scale).ravel()[0])
    alpha = (gam - eps) / sc
    assert alpha > 0
    k_sign = eps / alpha
    g = gam / alpha
    inv_s = 1.0 / sc

    xr = np.asarray(x, dtype=np.float32).reshape(M, IN) * np.float32(alpha)
    xT = np.ascontiguousarray(xr.T)                       # [IN, M]
    xT_blocked = np.ascontiguousarray(
        xT.reshape(NKB, KB, M).transpose(1, 0, 2)
    ).reshape(KB, NKB * M).astype(bfloat16)

    # un-center the integer codes: d = q + K*sign(q), exact in bf16
    wq = np.asarray(weight_q, dtype=np.float32)
    wd = (wq + np.float32(k_sign) * np.sign(wq)).astype(bfloat16)
    bias_bf = np.asarray(bias, dtype=np.float32).astype(bfloat16)

    in_maps = []
    for c in range(N_CORES):
        wTc = np.ascontiguousarray(
            wd[c * O_SH:(c + 1) * O_SH, :].T)             # [IN, O_SH]
        # pre-tile: row-block kp = its NH k-blocks side by side, so the
        # device loads each dequant tile as one contiguous DMA
        wTt = np.ascontiguousarray(
            wTc.reshape(NKB // NH, NH, KB, O_SH).transpose(0, 2, 1, 3)
        ).reshape(NKB // NH * KB, NH * O_SH)
        in_maps.append({
            "wT": wTt,
            "xT": xT_blocked,
            "bias": bias_bf[c * O_SH:(c + 1) * O_SH].reshape(1, O_SH),
        })
    return (inv_s, g), in_maps


def _run(nc, in_maps, **kw):
    from concourse import bass_utils
    return bass_utils.run_bass_kernel_spmd(
        nc, in_maps, core_ids=list(range(N_CORES)), **kw)


def kernel(x, epsilon, gamma, scale, bias, weight_q):
    consts, in_maps = _prep_inputs(x, epsilon, gamma, scale, bias, weight_q)
    nc = _get_nc(*consts)
    res = _run(nc, in_maps)
    out = np.concatenate(
        [np.asarray(res.results[c]["out"]) for c in range(N_CORES)], axis=1)
    return np.ascontiguousarray(out.reshape(B, S, OUT)).astype(np.float32)
